# revision 4
# baseline (speedup 1.0000x reference)
"""Trainium2 Bass kernel for a 2-layer weighted graph-attention network + bilinear
pair decoder, sharded row-wise (source-node dim) across 8 NeuronCores.

Self-contained: hardcodes all shapes from the problem spec.
  kernel(**inputs) -> np.ndarray   (full [100000] sigmoid probabilities)

Per-core program (core k owns node rows [512k, 512k+512)):
  h1 = x @ W1 (full, replicated);  hsrc/hdst via x @ (W1 @ a)  [associativity].
  Per row-block of 128: E = hdst_j broadcast via K=1 PE matmul; leaky-relu with
  per-partition bias hsrc_i on ACT (Prelu, alpha=0.2); P = exp(E') on ACT;
  denom = rowsum(P * adj) fused via scalar_tensor_tensor accumulate;
  B = P * (amount + count) (the sum computed by CCE accum-DMA during load);
  numerator = B @ h on PE (B transposed on-chip via PE transpose);
  out = elu(0.5 * numerator / denom).
  Layer outputs transposed on-chip + AllGathered; layer 2 repeats with W2.
  Decode: table = [h_final | h_final @ Wb] AllGathered to DRAM; per 128 pairs,
  indirect-DMA row gathers + fused multiply-reduce gives logits; sigmoid on ACT.
"""

import os
from contextlib import ExitStack

import numpy as np

import concourse.bass as bass
import concourse.bacc as bacc
import concourse.mybir as mybir
import concourse.tile as tile
from concourse.bass_utils import run_bass_kernel_spmd
from concourse.masks import make_identity

F32 = mybir.dt.float32
I32 = mybir.dt.int32
AF = mybir.ActivationFunctionType
OP = mybir.AluOpType

N = 4096          # nodes
DIN = 256         # input features
HID = 512         # hidden features
DOUT = 256        # output features
NPAIRS = 100000
NCORES = 8
R = N // NCORES   # rows per core = 512
NB = R // 128     # row blocks per core = 4
JH = 2048         # j-half width
PPC = 12544       # padded pairs per core = 128 * 98
PC = PPC // 128   # pair chunks = 98

_CACHE = {}
LAST_RESULTS = None  # BassKernelResults of the most recent run (for profiling)


def _attention_layer(nc, tc, ctx, *, D, h_sb, hsrc_col, er, adj_d, amt_d, cnt_d,
                     ident, ones_l, hout_sb, alpha_sb):
    """One weighted-GAT layer over this core's NB row blocks.

    h_sb:     [128, 32, D] SBUF, full h, natural layout (j on partitions per block)
    hsrc_col: [128, NB]  hsrc for this core's rows (partition = row within block)
    er:       [1, N]  hdst (full)
    """
    work = ctx.enter_context(tc.tile_pool(name=f"work{D}", bufs=2))
    btp = ctx.enter_context(tc.tile_pool(name=f"bt{D}", bufs=2))
    dpool = ctx.enter_context(tc.tile_pool(name=f"den{D}", bufs=2))
    fin = ctx.enter_context(tc.tile_pool(name=f"fin{D}", bufs=1))
    pe_ps = ctx.enter_context(tc.tile_pool(name=f"pe{D}", bufs=2, space="PSUM"))
    po_ps = ctx.enter_context(tc.tile_pool(name=f"po{D}", bufs=2, space="PSUM"))
    pt_ps = ctx.enter_context(tc.tile_pool(name=f"pt{D}", bufs=2, space="PSUM"))

    for m in range(NB):
        po = po_ps.tile([128, D], F32, tag="po", name=f"po_{D}_{m}")
        dparts = []
        for h in range(2):
            j0 = JH * h
            adj_t = work.tile([128, JH], F32, tag="adj", name=f"adj_{D}_{m}_{h}")
            nc.sync.dma_start(adj_t[:], adj_d[128 * m:128 * m + 128, j0:j0 + JH])
            s_t = work.tile([128, JH], F32, tag="s", name=f"s_{D}_{m}_{h}")
            nc.gpsimd.dma_start(s_t[:], amt_d[128 * m:128 * m + 128, j0:j0 + JH])
            nc.gpsimd.dma_start(s_t[:], cnt_d[128 * m:128 * m + 128, j0:j0 + JH],
                                accum_op=OP.add)

            L_t = work.tile([128, JH], F32, tag="L", name=f"L_{D}_{m}_{h}")
            A_t = work.tile([128, JH], F32, tag="A", name=f"A_{D}_{m}_{h}")
            for q in range(2):
                pe = pe_ps.tile([128, 1024], F32, tag="pe", name=f"pe_{D}_{m}_{h}_{q}")
                for r in range(2):
                    c0 = j0 + 1024 * q + 512 * r
                    nc.tensor.matmul(pe[:, 512 * r:512 * r + 512],
                                     lhsT=ones_l[:], rhs=er[:, c0:c0 + 512],
                                     start=True, stop=True)
                # E' = prelu(E + hsrc_i): per-partition bias, slope alpha
                nc.scalar.activation(L_t[:, 1024 * q:1024 * q + 1024], pe[:],
                                     AF.Prelu, bias=hsrc_col[:, m:m + 1],
                                     scale=1.0, alpha=alpha_sb[:])
            nc.scalar.activation(A_t[:], L_t[:], AF.Exp)

            # denom partial = sum_j A*adj (write target L_t is dead scratch)
            dpart = dpool.tile([128, 1], F32, tag="dh", name=f"dh_{D}_{m}_{h}")
            nc.vector.scalar_tensor_tensor(
                out=L_t[:], in0=A_t[:], scalar=1.0, in1=adj_t[:],
                op0=OP.mult, op1=OP.mult, accum_out=dpart[:])
            dparts.append(dpart)

            # B = A * s, in place over s
            nc.vector.tensor_mul(s_t[:], A_t[:], s_t[:])

            bt_t = btp.tile([128, 16, 128], F32, tag="bt", name=f"bt_{D}_{m}_{h}")
            for jq in range(4):
                ptp = pt_ps.tile([128, 512], F32, tag="pt", name=f"pt_{D}_{m}_{h}_{jq}")
                for r in range(4):
                    jc = 4 * jq + r
                    nc.tensor.transpose(ptp[:, 128 * r:128 * r + 128],
                                        s_t[:, 128 * jc:128 * jc + 128], ident[:])
                nc.any.tensor_copy(out=bt_t[:, 4 * jq:4 * jq + 4, :], in_=ptp[:])
            for jc in range(16):
                jg = 16 * h + jc
                nc.tensor.matmul(po[:], lhsT=bt_t[:, jc, :], rhs=h_sb[:, jg, :D],
                                 start=(h == 0 and jc == 0),
                                 stop=(h == 1 and jc == 15))

        den = dpool.tile([128, 1], F32, tag="den", name=f"den_{D}_{m}")
        nc.vector.tensor_add(den[:], dparts[0][:], dparts[1][:])
        rec = dpool.tile([128, 1], F32, tag="rec", name=f"rec_{D}_{m}")
        nc.vector.reciprocal(rec[:], den[:])
        rec2 = dpool.tile([128, 1], F32, tag="rec2", name=f"rec2_{D}_{m}")
        nc.vector.tensor_scalar_mul(rec2[:], rec[:], 0.5)
        # pre = 0.5 * numerator / denom ; then elu(pre) = max(pre, exp(min(pre,0))-1)
        pre_t = fin.tile([128, D], F32, tag="pre", name=f"pre_{D}_{m}", bufs=2)
        nc.scalar.activation(pre_t[:], po[:], AF.Copy, bias=0.0, scale=rec2[:])
        m1_t = fin.tile([128, D], F32, tag="m1", name=f"m1_{D}_{m}", bufs=2)
        nc.vector.tensor_scalar_min(m1_t[:], pre_t[:], 0.0)
        e1_t = fin.tile([128, D], F32, tag="e1", name=f"e1_{D}_{m}", bufs=2)
        nc.scalar.activation(e1_t[:], m1_t[:], AF.Exp)
        nc.vector.scalar_tensor_tensor(
            out=hout_sb[:, m, :], in0=e1_t[:], scalar=-1.0, in1=pre_t[:],
            op0=OP.add, op1=OP.max)


def _build():
    nc = bacc.Bacc("TRN2", target_bir_lowering=False, debug=False,
                   num_devices=NCORES)

    # ---- I/O ----
    xT_d = nc.dram_tensor("xT", [DIN, N], F32, kind="ExternalInput")
    xTl_d = nc.dram_tensor("xTl", [DIN, R], F32, kind="ExternalInput")
    adj_d = nc.dram_tensor("adj_s", [R, N], F32, kind="ExternalInput")
    amt_d = nc.dram_tensor("amt_s", [R, N], F32, kind="ExternalInput")
    cnt_d = nc.dram_tensor("cnt_s", [R, N], F32, kind="ExternalInput")
    W1_d = nc.dram_tensor("W1", [DIN, HID], F32, kind="ExternalInput")
    W1T_d = nc.dram_tensor("W1T", [HID, DIN], F32, kind="ExternalInput")
    W2_d = nc.dram_tensor("W2", [HID, DOUT], F32, kind="ExternalInput")
    W2T_d = nc.dram_tensor("W2T", [DOUT, HID], F32, kind="ExternalInput")
    Wb_d = nc.dram_tensor("Wb", [DOUT, DOUT], F32, kind="ExternalInput")
    # column 0 = a_dst, column 1 = a_src
    a1_d = nc.dram_tensor("a1", [HID, 2], F32, kind="ExternalInput")
    a2_d = nc.dram_tensor("a2", [DOUT, 2], F32, kind="ExternalInput")
    bb_d = nc.dram_tensor("bb", [1, 1], F32, kind="ExternalInput")
    pr_d = nc.dram_tensor("pairs_s", [128, 2 * PC], I32, kind="ExternalInput")
    out_d = nc.dram_tensor("out", [128, PC], F32, kind="ExternalOutput")

    with tile.TileContext(nc) as tc, ExitStack() as top:
        consts = top.enter_context(tc.tile_pool(name="consts", bufs=1))
        dram = top.enter_context(tc.tile_pool(name="dram", bufs=1, space="DRAM"))

        ident = consts.tile([128, 128], F32, name="ident")
        make_identity(nc, ident[:])
        alpha_sb = consts.tile([128, 1], F32, name="alpha_sb")
        nc.gpsimd.memset(alpha_sb[:], 0.2)
        ones_l = consts.tile([1, 128], F32, name="ones_l")
        nc.gpsimd.memset(ones_l[:], 1.0)

        W1_sb = consts.tile([128, 2, HID], F32, name="W1_sb")
        nc.sync.dma_start(W1_sb[:], W1_d[:].rearrange("(c p) h -> p c h", p=128))
        W1T_sb = consts.tile([128, 4, DIN], F32, name="W1T_sb")
        nc.sync.dma_start(W1T_sb[:], W1T_d[:].rearrange("(c p) h -> p c h", p=128))
        W2_sb = consts.tile([128, 4, DOUT], F32, name="W2_sb")
        nc.sync.dma_start(W2_sb[:], W2_d[:].rearrange("(c p) h -> p c h", p=128))
        W2T_sb = consts.tile([128, 2, HID], F32, name="W2T_sb")
        nc.sync.dma_start(W2T_sb[:], W2T_d[:].rearrange("(c p) h -> p c h", p=128))
        Wb_sb = consts.tile([128, 2, DOUT], F32, name="Wb_sb")
        nc.sync.dma_start(Wb_sb[:], Wb_d[:].rearrange("(c p) h -> p c h", p=128))
        a1_sb = consts.tile([128, 4, 2], F32, name="a1_sb")
        nc.sync.dma_start(a1_sb[:], a1_d[:].rearrange("(c p) t -> p c t", p=128))
        a2_sb = consts.tile([128, 2, 2], F32, name="a2_sb")
        nc.sync.dma_start(a2_sb[:], a2_d[:].rearrange("(c p) t -> p c t", p=128))

        # bb broadcast to [128, 1] via ones-matmul
        bb_sb = consts.tile([128, 1], F32, name="bb_sb")
        bb_row = consts.tile([1, 1], F32, name="bb_row")
        nc.sync.dma_start(bb_row[:], bb_d[:])
        with tc.tile_pool(name="pbb", bufs=1, space="PSUM") as pbb:
            bb_ps = pbb.tile([128, 1], F32, name="bb_ps")
            nc.tensor.matmul(bb_ps[:], lhsT=ones_l[:], rhs=bb_row[:],
                             start=True, stop=True)
            nc.any.tensor_copy(out=bb_sb[:], in_=bb_ps[:])

        hsrc1_col = consts.tile([128, NB], F32, name="hsrc1_col")
        hsrc2_col = consts.tile([128, NB], F32, name="hsrc2_col")
        hout1_pool = top.enter_context(tc.tile_pool(name="ho1p", bufs=1))
        hout1_sb = hout1_pool.tile([128, NB, HID], F32, name="hout1_sb")
        hout2_pool = top.enter_context(tc.tile_pool(name="ho2p", bufs=1))
        hout2_sb = hout2_pool.tile([128, NB, DOUT], F32, name="hout2_sb")

        # ================= scope 1: P1 + L1 =================
        with ExitStack() as sc1:
            s1c = sc1.enter_context(tc.tile_pool(name="s1c", bufs=1))
            h1_sb = s1c.tile([128, 32, HID], F32, name="h1_sb")
            er1 = s1c.tile([1, N], F32, name="er1")

            with ExitStack() as p1:
                p1c = p1.enter_context(tc.tile_pool(name="p1c", bufs=1))
                p1ps = p1.enter_context(tc.tile_pool(name="p1ps", bufs=1, space="PSUM"))
                xT_sb = p1c.tile([128, 2, N], F32, name="xT_sb")
                nc.sync.dma_start(xT_sb[:], xT_d[:].rearrange("(c p) n -> p c n", p=128))
                xTl_sb = p1c.tile([128, 2, R], F32, name="xTl_sb")
                nc.sync.dma_start(xTl_sb[:], xTl_d[:].rearrange("(c p) n -> p c n", p=128))

                # h1 = x @ W1 (full)
                for j in range(32):
                    pt = p1ps.tile([128, 512], F32, tag="h1ps", name=f"h1ps_{j}", bufs=4)
                    for ic in range(2):
                        nc.tensor.matmul(pt[:], lhsT=xT_sb[:, ic, 128 * j:128 * j + 128],
                                         rhs=W1_sb[:, ic, :],
                                         start=(ic == 0), stop=(ic == 1))
                    nc.any.tensor_copy(out=h1_sb[:, j, :], in_=pt[:])

                # v1 = [W1 @ a1d | W1 @ a1s] -> [128, ic, 2]
                v1_sb = p1c.tile([128, 2, 2], F32, name="v1_sb")
                for ic in range(2):
                    pv = p1ps.tile([128, 2], F32, tag="vps", name=f"v1ps_{ic}")
                    for hc in range(4):
                        nc.tensor.matmul(pv[:], lhsT=W1T_sb[:, hc, 128 * ic:128 * ic + 128],
                                         rhs=a1_sb[:, hc, :],
                                         start=(hc == 0), stop=(hc == 3))
                    nc.any.tensor_copy(out=v1_sb[:, ic, :], in_=pv[:])

                # er1 = hdst (full): row 0 of [v1d|v1s]^T @ xT
                for c8 in range(8):
                    ph = p1ps.tile([2, 512], F32, tag="hsd", name=f"hsd1ps_{c8}", bufs=2)
                    for ic in range(2):
                        nc.tensor.matmul(ph[:], lhsT=v1_sb[:, ic, :],
                                         rhs=xT_sb[:, ic, 512 * c8:512 * c8 + 512],
                                         start=(ic == 0), stop=(ic == 1))
                    nc.any.tensor_copy(out=er1[0:1, 512 * c8:512 * c8 + 512],
                                       in_=ph[0:1, :])

                # hsrc1_col[i, m] = sum_in x[i, in] * v1s[in]
                for m in range(NB):
                    pv = p1ps.tile([128, 1], F32, tag="hsc", name=f"hsc1_{m}")
                    for ic in range(2):
                        nc.tensor.matmul(pv[:], lhsT=xTl_sb[:, ic, 128 * m:128 * m + 128],
                                         rhs=v1_sb[:, ic, 1:2],
                                         start=(ic == 0), stop=(ic == 1))
                    nc.any.tensor_copy(out=hsrc1_col[:, m:m + 1], in_=pv[:])

            with ExitStack() as l1:
                _attention_layer(nc, tc, l1, D=HID, h_sb=h1_sb, hsrc_col=hsrc1_col,
                                 er=er1, adj_d=adj_d, amt_d=amt_d, cnt_d=cnt_d,
                                 ident=ident, ones_l=ones_l, hout_sb=hout1_sb,
                                 alpha_sb=alpha_sb)

        # ================= scope 2: T1 + L2 =================
        with ExitStack() as sc2:
            s2c = sc2.enter_context(tc.tile_pool(name="s2c", bufs=1))
            h2_sb = s2c.tile([128, 32, DOUT], F32, name="h2_sb")
            er2 = s2c.tile([1, N], F32, name="er2")

            with ExitStack() as t1:
                t1c = t1.enter_context(tc.tile_pool(name="t1c", bufs=1))
                t1ps = t1.enter_context(tc.tile_pool(name="t1ps", bufs=1, space="PSUM"))
                h1T_sb = t1c.tile([128, 4, R], F32, name="h1T_sb")
                for dc in range(4):
                    ptp = t1ps.tile([128, 512], F32, tag="t1t", name=f"h1t_{dc}", bufs=2)
                    for m in range(NB):
                        nc.tensor.transpose(ptp[:, 128 * m:128 * m + 128],
                                            hout1_sb[:, m, 128 * dc:128 * dc + 128],
                                            ident[:])
                    nc.any.tensor_copy(out=h1T_sb[:, dc, :], in_=ptp[:])

                ag1_in = dram.tile([HID, R], F32, name="ag1_in")
                ag1_out = dram.tile([NCORES * HID, R], F32, addr_space="Shared",
                                    name="ag1_out")
                nc.gpsimd.dma_start(
                    ag1_in[:].rearrange("(dc p) i -> p dc i", p=128), h1T_sb[:])
                nc.gpsimd.collective_compute(
                    "AllGather", OP.bypass, replica_groups=[list(range(NCORES))],
                    ins=[ag1_in.opt()], outs=[ag1_out.opt()])

                h1Tf_pool = t1.enter_context(tc.tile_pool(name="h1Tfp", bufs=1))
                h1Tf = h1Tf_pool.tile([128, 4, N], F32, name="h1Tf")
                ag1_view = ag1_out[:].rearrange("(r dc p) i -> dc p r i", r=NCORES,
                                                dc=4, p=128)
                for dc in range(4):
                    nc.sync.dma_start(h1Tf[:, dc, :], ag1_view[dc])

                # h2 = h1out @ W2 (full)
                for j in range(32):
                    pt = t1ps.tile([128, DOUT], F32, tag="h2ps", name=f"h2ps_{j}", bufs=2)
                    for dc in range(4):
                        nc.tensor.matmul(pt[:], lhsT=h1Tf[:, dc, 128 * j:128 * j + 128],
                                         rhs=W2_sb[:, dc, :],
                                         start=(dc == 0), stop=(dc == 3))
                    nc.any.tensor_copy(out=h2_sb[:, j, :], in_=pt[:])

                # v2 = [W2 @ a2d | W2 @ a2s]
                v2_sb = t1c.tile([128, 4, 2], F32, name="v2_sb")
                for dc in range(4):
                    pv = t1ps.tile([128, 2], F32, tag="v2ps", name=f"v2ps_{dc}")
                    for oc in range(2):
                        nc.tensor.matmul(pv[:], lhsT=W2T_sb[:, oc, 128 * dc:128 * dc + 128],
                                         rhs=a2_sb[:, oc, :],
                                         start=(oc == 0), stop=(oc == 1))
                    nc.any.tensor_copy(out=v2_sb[:, dc, :], in_=pv[:])

                # er2 = hdst2 (full)
                for c8 in range(8):
                    ph = t1ps.tile([2, 512], F32, tag="hsd2t", name=f"hsd2ps_{c8}", bufs=2)
                    for dc in range(4):
                        nc.tensor.matmul(ph[:], lhsT=v2_sb[:, dc, :],
                                         rhs=h1Tf[:, dc, 512 * c8:512 * c8 + 512],
                                         start=(dc == 0), stop=(dc == 3))
                    nc.any.tensor_copy(out=er2[0:1, 512 * c8:512 * c8 + 512],
                                       in_=ph[0:1, :])

                # hsrc2_col[i, m] = sum_d h1out[i, d] * v2s[d]  (local shard)
                for m in range(NB):
                    pv = t1ps.tile([128, 1], F32, tag="hsc2", name=f"hsc2_{m}")
                    for dc in range(4):
                        nc.tensor.matmul(pv[:], lhsT=h1T_sb[:, dc, 128 * m:128 * m + 128],
                                         rhs=v2_sb[:, dc, 1:2],
                                         start=(dc == 0), stop=(dc == 3))
                    nc.any.tensor_copy(out=hsrc2_col[:, m:m + 1], in_=pv[:])

            with ExitStack() as l2:
                _attention_layer(nc, tc, l2, D=DOUT, h_sb=h2_sb, hsrc_col=hsrc2_col,
                                 er=er2, adj_d=adj_d, amt_d=amt_d, cnt_d=cnt_d,
                                 ident=ident, ones_l=ones_l, hout_sb=hout2_sb,
                                 alpha_sb=alpha_sb)

        # ================= scope 3: T2 + decode =================
        ag2_out = dram.tile([N, 2 * DOUT], F32, addr_space="Shared", name="ag2_out")
        with ExitStack() as t2:
            t2c = t2.enter_context(tc.tile_pool(name="t2c", bufs=1))
            t2ps = t2.enter_context(tc.tile_pool(name="t2ps", bufs=1, space="PSUM"))
            tbl_sb = t2c.tile([128, NB, 2 * DOUT], F32, name="tbl_sb")
            hfT_sb = t2c.tile([128, 2, R], F32, name="hfT_sb")
            for m in range(NB):
                nc.any.tensor_copy(out=tbl_sb[:, m, 0:DOUT], in_=hout2_sb[:, m, :])
            for dc in range(2):
                ptp = t2ps.tile([128, 512], F32, tag="t2t", name=f"hft_{dc}", bufs=2)
                for m in range(NB):
                    nc.tensor.transpose(ptp[:, 128 * m:128 * m + 128],
                                        hout2_sb[:, m, 128 * dc:128 * dc + 128],
                                        ident[:])
                nc.any.tensor_copy(out=hfT_sb[:, dc, :], in_=ptp[:])
            for m in range(NB):
                pg = t2ps.tile([128, DOUT], F32, tag="gps", name=f"gps_{m}", bufs=2)
                for dc in range(2):
                    nc.tensor.matmul(pg[:], lhsT=hfT_sb[:, dc, 128 * m:128 * m + 128],
                                     rhs=Wb_sb[:, dc, :],
                                     start=(dc == 0), stop=(dc == 1))
                nc.any.tensor_copy(out=tbl_sb[:, m, DOUT:2 * DOUT], in_=pg[:])

            ag2_in = dram.tile([R, 2 * DOUT], F32, name="ag2_in")
            nc.gpsimd.dma_start(
                ag2_in[:].rearrange("(m p) d -> p m d", p=128), tbl_sb[:])
            nc.gpsimd.collective_compute(
                "AllGather", OP.bypass, replica_groups=[list(range(NCORES))],
                ins=[ag2_in.opt()], outs=[ag2_out.opt()])

        with ExitStack() as dec:
            dc_c = dec.enter_context(tc.tile_pool(name="dcc", bufs=1))
            zp = dec.enter_context(tc.tile_pool(name="zp", bufs=3))
            pairs_sb = dc_c.tile([128, 2 * PC], I32, name="pairs_sb")
            nc.sync.dma_start(pairs_sb[:], pr_d[:])
            logit_sb = dc_c.tile([128, PC], F32, name="logit_sb")
            for c in range(PC):
                zG = zp.tile([128, DOUT], F32, tag="zG", name=f"zG_{c}")
                nc.gpsimd.indirect_dma_start(
                    out=zG[:], out_offset=None, in_=ag2_out[:],
                    in_offset=bass.IndirectOffsetOnAxis(
                        ap=pairs_sb[:, 2 * c:2 * c + 1], axis=0),
                    element_offset=DOUT)
                zh = zp.tile([128, DOUT], F32, tag="zh", name=f"zh_{c}")
                nc.gpsimd.indirect_dma_start(
                    out=zh[:], out_offset=None, in_=ag2_out[:],
                    in_offset=bass.IndirectOffsetOnAxis(
                        ap=pairs_sb[:, 2 * c + 1:2 * c + 2], axis=0),
                    element_offset=0)
                scr = zp.tile([128, DOUT], F32, tag="scr", name=f"scr_{c}")
                nc.vector.scalar_tensor_tensor(
                    out=scr[:], in0=zG[:], scalar=1.0, in1=zh[:],
                    op0=OP.mult, op1=OP.mult,
                    accum_out=logit_sb[:, c:c + 1])
            prob_sb = dc_c.tile([128, PC], F32, name="prob_sb")
            nc.scalar.activation(prob_sb[:], logit_sb[:], AF.Sigmoid,
                                 bias=bb_sb[:], scale=1.0)
            nc.sync.dma_start(out_d[:], prob_sb[:])

    nc.compile()
    return nc


def kernel(**inputs):
    global LAST_RESULTS
    x = np.asarray(inputs["x"], dtype=np.float32)
    adj = np.asarray(inputs["adj"], dtype=np.float32)
    amount = np.asarray(inputs["amount"], dtype=np.float32)
    count = np.asarray(inputs["count"], dtype=np.float32)
    pairs = np.asarray(inputs["pairs"]).astype(np.int32)
    W1 = np.asarray(inputs["W1"], dtype=np.float32)
    a1s = np.asarray(inputs["a1_src"], dtype=np.float32)
    a1d = np.asarray(inputs["a1_dst"], dtype=np.float32)
    W2 = np.asarray(inputs["W2"], dtype=np.float32)
    a2s = np.asarray(inputs["a2_src"], dtype=np.float32)
    a2d = np.asarray(inputs["a2_dst"], dtype=np.float32)
    Wb = np.asarray(inputs["Wb"], dtype=np.float32)
    bb_in = np.asarray(inputs["bb"], dtype=np.float32).reshape(-1)
    bb = np.float32(bb_in[0]) if bb_in.size else np.float32(0.0)

    if "nc" not in _CACHE:
        _CACHE["nc"] = _build()
    nc = _CACHE["nc"]

    xT = np.ascontiguousarray(x.T)
    W1T = np.ascontiguousarray(W1.T)
    W2T = np.ascontiguousarray(W2.T)
    a1 = np.ascontiguousarray(np.stack([a1d, a1s], axis=1))  # col0=dst, col1=src
    a2 = np.ascontiguousarray(np.stack([a2d, a2s], axis=1))
    bb_arr = np.full((1, 1), bb, dtype=np.float32)

    in_maps = []
    per = NPAIRS // NCORES
    for k in range(NCORES):
        r0 = R * k
        pk = pairs[per * k: per * (k + 1)]
        pk_pad = np.zeros((PPC, 2), dtype=np.int32)
        pk_pad[:len(pk)] = pk
        in_maps.append(dict(
            xT=xT, xTl=np.ascontiguousarray(xT[:, r0:r0 + R]),
            adj_s=np.ascontiguousarray(adj[r0:r0 + R]),
            amt_s=np.ascontiguousarray(amount[r0:r0 + R]),
            cnt_s=np.ascontiguousarray(count[r0:r0 + R]),
            W1=W1, W1T=W1T, W2=W2, W2T=W2T, Wb=Wb, a1=a1, a2=a2, bb=bb_arr,
            pairs_s=np.ascontiguousarray(pk_pad.reshape(128, 2 * PC)),
        ))

    trace = os.environ.get("GAT_TRACE", "0") == "1"
    res = run_bass_kernel_spmd(nc, in_maps, core_ids=list(range(NCORES)),
                               trace=trace)
    LAST_RESULTS = res

    out = np.empty(NPAIRS, dtype=np.float32)
    for k in range(NCORES):
        out[per * k: per * (k + 1)] = res.results[k]["out"].reshape(-1)[:per]
    return out
